# revision 10
# baseline (speedup 1.0000x reference)
"""Causal self-attention (B=2, N=2048, E=1024, H=16, HD=64) on 8 trn2 NeuronCores.

Sharding: (batch, head-group) — core c handles batch c//4 and heads
4*(c%4) .. 4*(c%4)+3.  Each core computes its heads' QKV projections,
causal attention, and a partial out-projection over its 256 feature rows
of Wout; the host sums the 4 partials per batch and adds all biases that
are affine in the output (bout and the v-bias term, which is constant
because softmax rows sum to 1).

On-device layout avoids every transpose:
  - host feeds xT (E-major) so QK projections produce qT/kT [d, n] directly
  - S^T tiles [k, q] = kT-slice.T @ qT-slice (contraction over d)
  - exp on ScalarE (no max subtraction: logits are O(1) by construction)
  - PV uses expST tiles as lhsT with a ones-column appended to v to get the
    softmax denominator for free; normalization via gpsimd partition
    broadcast of the DVE reciprocal.
All matmuls run in float32r (full-rate fp32 mode, ~tf32 precision).
"""

import numpy as np

import concourse.bass as bass
import concourse.tile as tile
from concourse import bacc, mybir
from concourse import bass_utils

B, N, E, H = 2, 2048, 1024, 16
HD = 64
NCORES = 8
HPC = 4            # heads per core
NE = E // 128      # 8 e-chunks
NK = N // 128      # 16 k-tiles / n-chunks
NQ = N // 512      # 4 q-chunks
F32 = mybir.dt.float32
F32R = mybir.dt.float32r

_CACHE = {}


def _ceil_to(x, m):
    return ((x + m - 1) // m) * m


def _build_body(nc, tc, pools, dram, rep):
    """Emit one full kernel body. Tags are shared across repetitions."""
    xt_d, wq_d, wk_d, wv_d, wout_d, bqk_d, mask_d, ones_d, out_d = dram
    (pconst, pqk, pvext, psa, pesb, psmall, pout, ppsum_pv, ppsum) = pools

    # ---- constant loads -------------------------------------------------
    xt = []
    for e in range(NE):
        t = pconst.tile([128, N], F32R, tag=f"xt{e}")
        nc.sync.dma_start(t[:], xt_d[e])
        xt.append(t)
    wq_sb, wk_sb, wv_sb = [], [], []
    for e in range(NE):
        for lst, src, nm in ((wq_sb, wq_d, "wq"), (wk_sb, wk_d, "wk"),
                             (wv_sb, wv_d, "wv")):
            t = pconst.tile([128, 256], F32R, tag=f"{nm}{e}")
            nc.sync.dma_start(t[:], src[e])
            lst.append(t)
    wout_sb = []
    for p in range(2):
        t = pconst.tile([128, E], F32R, tag=f"wout{p}")
        nc.sync.dma_start(t[:], wout_d[p])
        wout_sb.append(t)
    bias_sb = {}
    for p in range(2):
        for i, nm in enumerate(("bq", "bk")):
            t = pconst.tile([128, 1], F32, tag=f"{nm}{p}")
            nc.sync.dma_start(t[:], bqk_d[p, i])
            bias_sb[(p, nm)] = t
    mask_sb = pconst.tile([128, 128], F32R, tag="mask")
    nc.sync.dma_start(mask_sb[:], mask_d)

    # ---- v projection: v_ext[nk] = [v_h0|1|v_h1|1|v_h2|1|v_h3|1] --------
    v_ext = []
    for nk in range(NK):
        vps = ppsum.tile([128, 256], F32, tag="gen")
        for e in range(NE):
            nc.tensor.matmul(vps[:], xt[e][:, nk * 128:(nk + 1) * 128],
                             wv_sb[e][:], start=(e == 0), stop=(e == NE - 1))
        vt = pvext.tile([128, 4 * 65], F32R, tag=f"vext{nk}")
        nc.sync.dma_start(
            vt[:].rearrange("p (h d) -> p h d", h=4)[:, :, 64:65],
            ones_d[:].rearrange("p (h d) -> p h d", h=4))
        nc.vector.tensor_copy(
            vt[:].rearrange("p (h d) -> p h d", h=4)[:, :, 0:64],
            vps[:].rearrange("p (h d) -> p h d", h=4))
        v_ext.append(vt)

    # ---- qk projections (both pairs) ------------------------------------
    qT = {}
    kT = {}
    for p in range(2):
        for nm, w_sb, bias in (("q", wq_sb, bias_sb[(p, "bq")]),
                               ("k", wk_sb, bias_sb[(p, "bk")])):
            dst = pqk.tile([128, N], F32R, tag=f"{nm}T{p}")
            for nq in range(NQ):
                ps = ppsum.tile([128, 512], F32, tag="gen")
                for e in range(NE):
                    nc.tensor.matmul(
                        ps[:], w_sb[e][:, p * 128:(p + 1) * 128],
                        xt[e][:, nq * 512:(nq + 1) * 512],
                        start=(e == 0), stop=(e == NE - 1))
                nc.vector.tensor_scalar_add(
                    dst[:, nq * 512:(nq + 1) * 512], ps[:], bias[:])
            (qT if nm == "q" else kT)[p] = dst

    # ---- attention, one head at a time (k-major over key tiles) ---------
    saT = {}
    for p in range(2):
        saT[p] = psa.tile([128, N], F32R, tag=f"saT{p}", name=f"saT{p}")
    for p in range(2):
        for hh in range(2):
            hloc = 2 * p + hh           # head index within core (0..3)
            rb = hh * 64                # partition row base in qT/kT pair tiles
            pv = [ppsum_pv.tile([65, 512], F32, tag="pv", name=f"pv{i}")
                  for i in range(NQ)]
            for kj in range(NK):
                q0 = 128 * kj
                # scores^T for keys [q0, q0+128) vs queries [q0, N)
                esb = pesb.tile([128, N], F32R, tag="esb")
                c0 = q0
                while c0 < N:
                    w = min(1024, N - c0)
                    sps = ppsum.tile([128, w], F32, tag="gen")
                    off = 0
                    while off < w:
                        n = min(512, w - off)
                        nc.tensor.matmul(
                            sps[:, off:off + n],
                            kT[p][rb:rb + 64, q0:q0 + 128],
                            qT[p][rb:rb + 64, c0 + off:c0 + off + n],
                            start=True, stop=True)
                        off += n
                    nc.scalar.activation(esb[:, c0:c0 + w], sps[:],
                                         mybir.ActivationFunctionType.Exp)
                    c0 += w
                # causal mask on the diagonal 128x128 block
                nc.vector.tensor_mul(esb[:, q0:q0 + 128],
                                     esb[:, q0:q0 + 128], mask_sb[:])
                # accumulate PV for every query chunk that sees this k-tile
                for qi in range(kj // 4, NQ):
                    # first valid query column for this k-tile is q0; the
                    # columns of esb below it were never written
                    s = max(qi * 512, q0)
                    nc.tensor.matmul(
                        pv[qi][:, s - qi * 512:512],
                        v_ext[kj][:, hloc * 65:hloc * 65 + 65],
                        esb[:, s:(qi + 1) * 512],
                        start=(kj == 0), stop=(kj == 4 * qi + 3))
                    if kj == 4 * qi + 3:
                        rcp = psmall.tile([1, 512], F32, tag="rcp")
                        nc.vector.reciprocal(rcp[:], pv[qi][64:65, :])
                        bc = psmall.tile([64, 512], F32, tag="bc")
                        nc.gpsimd.partition_broadcast(bc[:], rcp[:])
                        nc.vector.tensor_mul(
                            saT[p][rb:rb + 64, qi * 512:(qi + 1) * 512],
                            pv[qi][0:64, :], bc[:])

    # ---- out projection (partial over this core's 256 features) ---------
    for nk in range(NK):
        ot = pout.tile([128, E], F32, tag="outsb")
        for oc in range(2):
            ps = ppsum.tile([128, 512], F32, tag="gen")
            for p in range(2):
                nc.tensor.matmul(ps[:],
                                 saT[p][:, nk * 128:(nk + 1) * 128],
                                 wout_sb[p][:, oc * 512:(oc + 1) * 512],
                                 start=(p == 0), stop=(p == 1))
            if oc == 0:
                nc.scalar.copy(ot[:, oc * 512:(oc + 1) * 512], ps[:])
            else:
                nc.vector.tensor_copy(ot[:, oc * 512:(oc + 1) * 512], ps[:])
        nc.sync.dma_start(out_d[nk], ot[:])


def build_nc(reps=1):
    nc = bacc.Bacc("TRN2", target_bir_lowering=False, debug=False,
                   enable_asserts=True, num_devices=NCORES)
    xt_d = nc.dram_tensor("xt", [NE, 128, N], F32R, kind="ExternalInput").ap()
    wq_d = nc.dram_tensor("wq", [NE, 128, 256], F32R, kind="ExternalInput").ap()
    wk_d = nc.dram_tensor("wk", [NE, 128, 256], F32R, kind="ExternalInput").ap()
    wv_d = nc.dram_tensor("wv", [NE, 128, 256], F32R, kind="ExternalInput").ap()
    wout_d = nc.dram_tensor("wout", [2, 128, E], F32R, kind="ExternalInput").ap()
    bqk_d = nc.dram_tensor("bqk", [2, 2, 128, 1], F32, kind="ExternalInput").ap()
    mask_d = nc.dram_tensor("mask", [128, 128], F32R, kind="ExternalInput").ap()
    ones_d = nc.dram_tensor("ones", [128, 4], F32R, kind="ExternalInput").ap()
    out_d = nc.dram_tensor("out", [NK, 128, E], F32, kind="ExternalOutput").ap()
    dram = (xt_d, wq_d, wk_d, wv_d, wout_d, bqk_d, mask_d, ones_d, out_d)

    with tile.TileContext(nc) as tc:
        from contextlib import ExitStack
        with ExitStack() as ctx:
            pconst = ctx.enter_context(tc.tile_pool(name="const", bufs=1))
            pqk = ctx.enter_context(tc.tile_pool(name="qk", bufs=1))
            pvext = ctx.enter_context(tc.tile_pool(name="vext", bufs=1))
            psa = ctx.enter_context(tc.tile_pool(name="sa", bufs=1))
            pesb = ctx.enter_context(tc.tile_pool(name="esb", bufs=3))
            psmall = ctx.enter_context(tc.tile_pool(name="small", bufs=2))
            pout = ctx.enter_context(tc.tile_pool(name="outsb", bufs=2))
            ppsum_pv = ctx.enter_context(
                tc.tile_pool(name="pvps", bufs=4, space="PSUM"))
            ppsum = ctx.enter_context(
                tc.tile_pool(name="gps", bufs=2, space="PSUM"))
            pools = (pconst, pqk, pvext, psa, pesb, psmall, pout,
                     ppsum_pv, ppsum)
            for r in range(reps):
                _build_body(nc, tc, pools, dram, r)
    nc.compile()
    return nc


def make_in_maps(x, Wqkv, bqkv, Wout):
    """Per-core input dicts. Shapes per reference: x[B,N,E], Wqkv[H,E,3HD],
    bqkv[H,3HD], Wout[E,E].  Split: cols 0:64=k, 64:128=q, 128:192=v."""
    Wk = Wqkv[:, :, 0:HD]
    Wq = Wqkv[:, :, HD:2 * HD] * (1.0 / np.sqrt(HD))
    Wv = Wqkv[:, :, 2 * HD:3 * HD]
    bk = bqkv[:, 0:HD]
    bq = bqkv[:, HD:2 * HD] * (1.0 / np.sqrt(HD))

    # expS^T tile rows are k, cols are q: keep k <= q -> upper triangular
    mask = np.triu(np.ones((128, 128), dtype=np.float32))
    in_maps = []
    for c in range(NCORES):
        b, hg = divmod(c, 4)
        hs = slice(4 * hg, 4 * hg + 4)

        xT = np.ascontiguousarray(x[b].T).reshape(NE, 128, N)

        def pack(w):  # [4,E,64] -> [NE,128,256]
            return np.ascontiguousarray(
                w.reshape(4, NE, 128, HD).transpose(1, 2, 0, 3)
                 .reshape(NE, 128, 256))

        wq = pack(Wq[hs])
        wk = pack(Wk[hs])
        wv = pack(Wv[hs])
        wout = np.ascontiguousarray(
            Wout[4 * hg * HD:(4 * hg + 4) * HD].reshape(2, 128, E))
        bqk = np.stack([
            np.stack([bq[4 * hg + 2 * p:4 * hg + 2 * p + 2].reshape(128),
                      bk[4 * hg + 2 * p:4 * hg + 2 * p + 2].reshape(128)])
            for p in range(2)]).reshape(2, 2, 128, 1)
        in_maps.append({
            "xt": xT.astype(np.float32),
            "wq": wq.astype(np.float32), "wk": wk.astype(np.float32),
            "wv": wv.astype(np.float32),
            "wout": wout.astype(np.float32),
            "bqk": bqk.astype(np.float32),
            "mask": mask,
            "ones": np.ones((128, 4), dtype=np.float32),
        })
    return in_maps


def combine(results, bqkv, Wout, bout):
    bv = bqkv[:, 2 * HD:3 * HD].reshape(E)          # concat over heads
    const_row = bv @ Wout + bout                     # [E]
    out = np.zeros((B, N, E), dtype=np.float32)
    for c in range(NCORES):
        b = c // 4
        out[b] += results[c]["out"].reshape(N, E)
    out += const_row[None, None, :].astype(np.float32)
    return out


def kernel(x, Wqkv, bqkv, Wout, bout):
    x = np.asarray(x, dtype=np.float32)
    Wqkv = np.asarray(Wqkv, dtype=np.float32)
    bqkv = np.asarray(bqkv, dtype=np.float32)
    Wout = np.asarray(Wout, dtype=np.float32)
    bout = np.asarray(bout, dtype=np.float32)

    if "nc" not in _CACHE:
        _CACHE["nc"] = build_nc(reps=1)
    nc = _CACHE["nc"]
    in_maps = make_in_maps(x, Wqkv, bqkv, Wout)
    res = bass_utils.run_bass_kernel_spmd(
        nc, in_maps, core_ids=list(range(NCORES)), trace=False)
    return combine(res.results, bqkv, Wout, bout)


# revision 12
# speedup vs baseline: 165.9973x; 165.9973x over previous
"""Causal self-attention (B=2, N=2048, E=1024, H=16, HD=64) on 8 trn2 NeuronCores.

Sharding: (batch, head-group) — core c handles batch c//4 and heads
4*(c%4) .. 4*(c%4)+3.  Each core computes its heads' QKV projections,
causal attention, and a partial out-projection over its 256 feature rows
of Wout; the host sums the 4 partials per batch and adds all biases that
are affine in the output (bout and the v-bias term, which is constant
because softmax rows sum to 1).

On-device layout avoids every transpose:
  - host feeds xT (E-major) so QK projections produce qT/kT [d, n] directly
  - S^T tiles [k, q] = kT-slice.T @ qT-slice (contraction over d)
  - exp on ScalarE (no max subtraction: logits are O(1) by construction)
  - PV uses expST tiles as lhsT with a ones-column appended to v to get the
    softmax denominator for free; normalization via gpsimd partition
    broadcast of the DVE reciprocal.
All matmuls run in float32r (full-rate fp32 mode, ~tf32 precision).
"""

import numpy as np

import concourse.bass as bass
import concourse.tile as tile
from concourse import bacc, mybir
from concourse import bass_utils

B, N, E, H = 2, 2048, 1024, 16
HD = 64
NCORES = 8
HPC = 4            # heads per core
NE = E // 128      # 8 e-chunks
NK = N // 128      # 16 k-tiles / n-chunks
NQ = N // 512      # 4 q-chunks
F32 = mybir.dt.float32
F32R = mybir.dt.float32r

_CACHE = {}


def _ceil_to(x, m):
    return ((x + m - 1) // m) * m


def _build_body(nc, tc, pools, dram, rep):
    """Emit one full kernel body. Tags are shared across repetitions."""
    xt_d, wq_d, wk_d, wv_d, wout_d, bqk_d, mask_d, ones_d, out_d = dram
    (pconst, pqk, pvext, psa, pesb, psmall, pout, ppsum_pv, ppsum) = pools

    # ---- constant loads -------------------------------------------------
    xt = []
    for e in range(NE):
        t = pconst.tile([128, N], F32R, tag=f"xt{e}")
        nc.sync.dma_start(t[:], xt_d[e])
        xt.append(t)
    wq_sb, wk_sb, wv_sb = [], [], []
    for e in range(NE):
        for lst, src, nm in ((wq_sb, wq_d, "wq"), (wk_sb, wk_d, "wk"),
                             (wv_sb, wv_d, "wv")):
            t = pconst.tile([128, 256], F32R, tag=f"{nm}{e}")
            nc.sync.dma_start(t[:], src[e])
            lst.append(t)
    wout_sb = []
    for p in range(2):
        t = pconst.tile([128, E], F32R, tag=f"wout{p}")
        nc.sync.dma_start(t[:], wout_d[p])
        wout_sb.append(t)
    bias_sb = {}
    for p in range(2):
        for i, nm in enumerate(("bq", "bk")):
            t = pconst.tile([128, 1], F32, tag=f"{nm}{p}")
            nc.sync.dma_start(t[:], bqk_d[p, i])
            bias_sb[(p, nm)] = t
    mask_sb = pconst.tile([128, 128], F32R, tag="mask")
    nc.sync.dma_start(mask_sb[:], mask_d)

    # ---- v projection: v_ext[nk] = [v_h0|1|v_h1|1|v_h2|1|v_h3|1] --------
    v_ext = []
    for nk in range(NK):
        vps = ppsum.tile([128, 256], F32, tag="gen")
        for e in range(NE):
            nc.tensor.matmul(vps[:], xt[e][:, nk * 128:(nk + 1) * 128],
                             wv_sb[e][:], start=(e == 0), stop=(e == NE - 1))
        vt = pvext.tile([128, 4 * 65], F32R, tag=f"vext{nk}")
        nc.sync.dma_start(
            vt[:].rearrange("p (h d) -> p h d", h=4)[:, :, 64:65],
            ones_d[:].rearrange("p (h d) -> p h d", h=4))
        nc.vector.tensor_copy(
            vt[:].rearrange("p (h d) -> p h d", h=4)[:, :, 0:64],
            vps[:].rearrange("p (h d) -> p h d", h=4))
        v_ext.append(vt)

    # ---- qk projections (both pairs) ------------------------------------
    qT = {}
    kT = {}
    for p in range(2):
        for nm, w_sb, bias in (("q", wq_sb, bias_sb[(p, "bq")]),
                               ("k", wk_sb, bias_sb[(p, "bk")])):
            dst = pqk.tile([128, N], F32R, tag=f"{nm}T{p}")
            for nq in range(NQ):
                ps = ppsum.tile([128, 512], F32, tag="gen")
                for e in range(NE):
                    nc.tensor.matmul(
                        ps[:], w_sb[e][:, p * 128:(p + 1) * 128],
                        xt[e][:, nq * 512:(nq + 1) * 512],
                        start=(e == 0), stop=(e == NE - 1))
                nc.vector.tensor_scalar_add(
                    dst[:, nq * 512:(nq + 1) * 512], ps[:], bias[:])
            (qT if nm == "q" else kT)[p] = dst

    # ---- attention, one head at a time (k-major over key tiles) ---------
    saT = {}
    for p in range(2):
        saT[p] = psa.tile([128, N], F32R, tag=f"saT{p}", name=f"saT{p}")
    for p in range(2):
        for hh in range(2):
            hloc = 2 * p + hh           # head index within core (0..3)
            rb = hh * 64                # partition row base in qT/kT pair tiles
            pv = [ppsum_pv.tile([65, 512], F32, tag="pv", name=f"pv{i}")
                  for i in range(NQ)]
            for kj in range(NK):
                q0 = 128 * kj
                # scores^T for keys [q0, q0+128) vs queries [q0, N)
                esb = pesb.tile([128, N], F32R, tag="esb")
                c0 = q0
                while c0 < N:
                    w = min(1024, N - c0)
                    sps = ppsum.tile([128, w], F32, tag="gen")
                    off = 0
                    while off < w:
                        n = min(512, w - off)
                        nc.tensor.matmul(
                            sps[:, off:off + n],
                            kT[p][rb:rb + 64, q0:q0 + 128],
                            qT[p][rb:rb + 64, c0 + off:c0 + off + n],
                            start=True, stop=True)
                        off += n
                    nc.scalar.activation(esb[:, c0:c0 + w], sps[:],
                                         mybir.ActivationFunctionType.Exp)
                    c0 += w
                # causal mask on the diagonal 128x128 block
                nc.vector.tensor_mul(esb[:, q0:q0 + 128],
                                     esb[:, q0:q0 + 128], mask_sb[:])
                # accumulate PV for every query chunk that sees this k-tile
                for qi in range(kj // 4, NQ):
                    # first valid query column for this k-tile is q0; the
                    # columns of esb below it were never written
                    s = max(qi * 512, q0)
                    nc.tensor.matmul(
                        pv[qi][:, s - qi * 512:512],
                        v_ext[kj][:, hloc * 65:hloc * 65 + 65],
                        esb[:, s:(qi + 1) * 512],
                        start=(kj == 0), stop=(kj == 4 * qi + 3))
                    if kj == 4 * qi + 3:
                        rcp = psmall.tile([1, 512], F32, tag="rcp")
                        nc.vector.reciprocal(rcp[:], pv[qi][64:65, :])
                        bc = psmall.tile([64, 512], F32, tag="bc")
                        nc.gpsimd.partition_broadcast(bc[:], rcp[:])
                        nc.vector.tensor_mul(
                            saT[p][rb:rb + 64, qi * 512:(qi + 1) * 512],
                            pv[qi][0:64, :], bc[:])

    # ---- out projection (partial over this core's 256 features) ---------
    for nk in range(NK):
        ot = pout.tile([128, E], F32, tag="outsb")
        for oc in range(2):
            ps = ppsum.tile([128, 512], F32, tag="gen")
            for p in range(2):
                nc.tensor.matmul(ps[:],
                                 saT[p][:, nk * 128:(nk + 1) * 128],
                                 wout_sb[p][:, oc * 512:(oc + 1) * 512],
                                 start=(p == 0), stop=(p == 1))
            if oc == 0:
                nc.scalar.copy(ot[:, oc * 512:(oc + 1) * 512], ps[:])
            else:
                nc.vector.tensor_copy(ot[:, oc * 512:(oc + 1) * 512], ps[:])
        nc.sync.dma_start(out_d[nk], ot[:])


def build_nc(reps=1, loop=None):
    nc = bacc.Bacc("TRN2", target_bir_lowering=False, debug=False,
                   enable_asserts=True, num_devices=NCORES)
    xt_d = nc.dram_tensor("xt", [NE, 128, N], F32R, kind="ExternalInput").ap()
    wq_d = nc.dram_tensor("wq", [NE, 128, 256], F32R, kind="ExternalInput").ap()
    wk_d = nc.dram_tensor("wk", [NE, 128, 256], F32R, kind="ExternalInput").ap()
    wv_d = nc.dram_tensor("wv", [NE, 128, 256], F32R, kind="ExternalInput").ap()
    wout_d = nc.dram_tensor("wout", [2, 128, E], F32R, kind="ExternalInput").ap()
    bqk_d = nc.dram_tensor("bqk", [2, 2, 128, 1], F32, kind="ExternalInput").ap()
    mask_d = nc.dram_tensor("mask", [128, 128], F32R, kind="ExternalInput").ap()
    ones_d = nc.dram_tensor("ones", [128, 4], F32R, kind="ExternalInput").ap()
    out_d = nc.dram_tensor("out", [NK, 128, E], F32, kind="ExternalOutput").ap()
    dram = (xt_d, wq_d, wk_d, wv_d, wout_d, bqk_d, mask_d, ones_d, out_d)

    with tile.TileContext(nc) as tc:
        from contextlib import ExitStack
        with ExitStack() as ctx:
            pconst = ctx.enter_context(tc.tile_pool(name="const", bufs=1))
            pqk = ctx.enter_context(tc.tile_pool(name="qk", bufs=1))
            pvext = ctx.enter_context(tc.tile_pool(name="vext", bufs=1))
            psa = ctx.enter_context(tc.tile_pool(name="sa", bufs=1))
            pesb = ctx.enter_context(tc.tile_pool(name="esb", bufs=3))
            psmall = ctx.enter_context(tc.tile_pool(name="small", bufs=2))
            pout = ctx.enter_context(tc.tile_pool(name="outsb", bufs=2))
            ppsum_pv = ctx.enter_context(
                tc.tile_pool(name="pvps", bufs=4, space="PSUM"))
            ppsum = ctx.enter_context(
                tc.tile_pool(name="gps", bufs=2, space="PSUM"))
            pools = (pconst, pqk, pvext, psa, pesb, psmall, pout,
                     ppsum_pv, ppsum)
            if loop is not None:
                with tc.For_i(0, loop, 1,
                              hint_engines=(mybir.EngineType.PE,
                                            mybir.EngineType.Activation,
                                            mybir.EngineType.DVE,
                                            mybir.EngineType.SP)):
                    _build_body(nc, tc, pools, dram, 0)
            else:
                for r in range(reps):
                    _build_body(nc, tc, pools, dram, r)
    nc.compile()
    return nc


def make_in_maps(x, Wqkv, bqkv, Wout):
    """Per-core input dicts. Shapes per reference: x[B,N,E], Wqkv[H,E,3HD],
    bqkv[H,3HD], Wout[E,E].  Split: cols 0:64=k, 64:128=q, 128:192=v."""
    Wk = Wqkv[:, :, 0:HD]
    Wq = Wqkv[:, :, HD:2 * HD] * (1.0 / np.sqrt(HD))
    Wv = Wqkv[:, :, 2 * HD:3 * HD]
    bk = bqkv[:, 0:HD]
    bq = bqkv[:, HD:2 * HD] * (1.0 / np.sqrt(HD))

    # expS^T tile rows are k, cols are q: keep k <= q -> upper triangular
    mask = np.triu(np.ones((128, 128), dtype=np.float32))
    in_maps = []
    for c in range(NCORES):
        b, hg = divmod(c, 4)
        hs = slice(4 * hg, 4 * hg + 4)

        xT = np.ascontiguousarray(x[b].T).reshape(NE, 128, N)

        def pack(w):  # [4,E,64] -> [NE,128,256]
            return np.ascontiguousarray(
                w.reshape(4, NE, 128, HD).transpose(1, 2, 0, 3)
                 .reshape(NE, 128, 256))

        wq = pack(Wq[hs])
        wk = pack(Wk[hs])
        wv = pack(Wv[hs])
        wout = np.ascontiguousarray(
            Wout[4 * hg * HD:(4 * hg + 4) * HD].reshape(2, 128, E))
        bqk = np.stack([
            np.stack([bq[4 * hg + 2 * p:4 * hg + 2 * p + 2].reshape(128),
                      bk[4 * hg + 2 * p:4 * hg + 2 * p + 2].reshape(128)])
            for p in range(2)]).reshape(2, 2, 128, 1)
        in_maps.append({
            "xt": xT.astype(np.float32),
            "wq": wq.astype(np.float32), "wk": wk.astype(np.float32),
            "wv": wv.astype(np.float32),
            "wout": wout.astype(np.float32),
            "bqk": bqk.astype(np.float32),
            "mask": mask,
            "ones": np.ones((128, 4), dtype=np.float32),
        })
    return in_maps


def combine(results, bqkv, Wout, bout):
    bv = bqkv[:, 2 * HD:3 * HD].reshape(E)          # concat over heads
    const_row = bv @ Wout + bout                     # [E]
    out = np.zeros((B, N, E), dtype=np.float32)
    for c in range(NCORES):
        b = c // 4
        out[b] += results[c]["out"].reshape(N, E)
    out += const_row[None, None, :].astype(np.float32)
    return out


def kernel(x, Wqkv, bqkv, Wout, bout):
    x = np.asarray(x, dtype=np.float32)
    Wqkv = np.asarray(Wqkv, dtype=np.float32)
    bqkv = np.asarray(bqkv, dtype=np.float32)
    Wout = np.asarray(Wout, dtype=np.float32)
    bout = np.asarray(bout, dtype=np.float32)

    if "nc" not in _CACHE:
        _CACHE["nc"] = build_nc(reps=1)
    nc = _CACHE["nc"]
    in_maps = make_in_maps(x, Wqkv, bqkv, Wout)
    res = bass_utils.run_bass_kernel_spmd(
        nc, in_maps, core_ids=list(range(NCORES)), trace=False)
    return combine(res.results, bqkv, Wout, bout)
